# revision 1
# baseline (speedup 1.0000x reference)
"""ChannelAttention Trainium2 Bass kernel.

Full (unsharded) inputs -> full output. Data-parallel over batch B=8 across
the 8 NeuronCores (one batch element per core, SPMD program, no collectives).

Per-core math (N=4096 tokens, C=512 channels):
    qkv = x @ Wqkv + bqkv ; q,k,v = relu(split(qkv))
    scores = (q^T k) / sqrt(C)           # [C, C] contraction over tokens
    attn = softmax(scores, -1) * adj
    y = v @ attn ; out = y @ Wo + bo

Matmuls run in float32r (fp32 storage, ~1e-4 rel err, full PE rate).
"""

import sys

sys.path.insert(0, "/opt/trn_rl_repo")

from contextlib import ExitStack

import numpy as np

import concourse.bass as bass
import concourse.mybir as mybir
import concourse.tile as tile
from concourse import bacc
from concourse.bass import ds, ts
from concourse.bass_utils import run_bass_kernel_spmd
from concourse.masks import make_identity

# Problem shape (hardcoded per contract).
B, N, C = 8, 4096, 512
P = 128
CC = C // P            # channel chunks (4)
NT = N // P            # token tiles (32)
TPS = 4                # token tiles per slab
NS = NT // TPS         # slabs (8)
SLAB = TPS * P         # tokens per slab (512)

F32 = mybir.dt.float32
F32R = mybir.dt.float32r

_CACHE = {}


def build(reps: int = 1, mm_dt=None, tp_bufs=2, proj_bufs=2, qk_bufs=3,
          xin_bufs=3, xtp_bufs=2, y_bufs=4, pipe_p2=True):
    MMD = F32R if mm_dt is None else mm_dt
    nc = bacc.Bacc("TRN2", target_bir_lowering=False, debug=False, num_devices=8)

    x = nc.dram_tensor("x", [N, C], F32, kind="ExternalInput").ap()
    adj = nc.dram_tensor("adj", [C, C], F32, kind="ExternalInput").ap()
    wqkv = nc.dram_tensor("Wqkv", [C, 3 * C], F32, kind="ExternalInput").ap()
    bqkv = nc.dram_tensor("bqkv", [3 * C], F32, kind="ExternalInput").ap()
    wo = nc.dram_tensor("Wo", [C, C], F32, kind="ExternalInput").ap()
    bo = nc.dram_tensor("bo", [C], F32, kind="ExternalInput").ap()
    out = nc.dram_tensor("out", [N, C], F32, kind="ExternalOutput").ap()

    s = 1.0 / float(np.sqrt(C))

    with tile.TileContext(nc) as tc, ExitStack() as ctx:
        const = ctx.enter_context(tc.tile_pool(name="const", bufs=1))

        # ---- constants -------------------------------------------------
        with tc.tile_pool(name="stage", bufs=1) as stage:
            wqkv_f = stage.tile([P, CC, 3 * C], F32, tag="stage_wqkv")
            nc.sync.dma_start(wqkv_f[:], wqkv.rearrange("(o p) d -> p o d", p=P))
            wqkv_r = const.tile([P, CC, 3 * C], MMD)
            nc.vector.tensor_copy(wqkv_r[:], wqkv_f[:])

            wo_f = stage.tile([P, CC, C], F32, tag="stage_wo")
            nc.sync.dma_start(wo_f[:], wo.rearrange("(o p) d -> p o d", p=P))
            wo_r = const.tile([P, CC, C], MMD)
            nc.vector.tensor_copy(wo_r[:], wo_f[:])

            brow_f = stage.tile([1, 3 * C], F32, tag="stage_b")
            nc.sync.dma_start(brow_f[:], bqkv[None, :])
            brow_r = const.tile([1, 3 * C], MMD)
            nc.vector.tensor_copy(brow_r[:], brow_f[:])

            borow_f = stage.tile([1, C], F32, tag="stage_bo")
            nc.sync.dma_start(borow_f[:], bo[None, :])
            borow_r = const.tile([1, C], MMD)
            nc.vector.tensor_copy(borow_r[:], borow_f[:])

            ones_f = stage.tile([1, P], F32, tag="stage_ones")
            nc.gpsimd.memset(ones_f[:], 1.0)
            ones_r = const.tile([1, P], MMD)
            nc.vector.tensor_copy(ones_r[:], ones_f[:])

        # v-bias, per-partition layout [p, chunk]
        bv = const.tile([P, CC], F32)
        nc.sync.dma_start(bv[:], bqkv[2 * C :].rearrange("(o p) -> p o", p=P))

        ident = const.tile([P, P], F32)
        make_identity(nc, ident[:])

        adj_sb = const.tile([P, CC, C], F32)
        nc.sync.dma_start(adj_sb[:], adj.rearrange("(o p) d -> p o d", p=P))

        vt_sb = const.tile([P, CC, N], MMD)      # v^T, channel-major
        attn_sb = const.tile([P, CC, C], MMD)    # gated softmax rows

        # bo broadcast to [P, C] once (fold bias into pass-2 evacuation)
        bo_bc = const.tile([P, C], F32)
        with tc.tile_pool(name="bo_ps", bufs=1, space="PSUM") as bo_ps_pool:
            bo_ps = bo_ps_pool.tile([P, C], F32, name="bo_ps")
            nc.tensor.matmul(bo_ps[:], ones_r[:], borow_r[:], start=True, stop=True)
            nc.vector.tensor_copy(bo_bc[:], bo_ps[:])

        # ---- pass 1: qkv projection + channel scores -------------------
        scores_pool = ctx.enter_context(
            tc.tile_pool(name="scores", bufs=1, space="PSUM")
        )
        scores_ps = [
            scores_pool.tile([P, C], F32, tag=f"scores{o}", name=f"scores{o}")
            for o in range(CC)
        ]

        rep_ctx = tc.For_i(0, reps, 1) if reps > 1 else None
        if rep_ctx is not None:
            ctx.enter_context(rep_ctx)

        with (
            tc.tile_pool(name="tp_ps", bufs=tp_bufs, space="PSUM") as tp_ps,
            tc.tile_pool(name="proj_ps", bufs=proj_bufs, space="PSUM") as proj_ps,
            tc.tile_pool(name="xin", bufs=xin_bufs) as xin,
            tc.tile_pool(name="xtp", bufs=xtp_bufs) as xtp,
            tc.tile_pool(name="qk", bufs=qk_bufs) as qk,
        ):
            for sl in range(NS):
                xt_slab = xtp.tile([P, CC, SLAB], MMD, tag="xT")
                for tt in range(TPS):
                    t = sl * TPS + tt
                    x_t = xin.tile([P, C], F32, tag="x")
                    nc.sync.dma_start(x_t[:], x[ts(t, P), :])

                    # transpose 128x512 -> xT chunks via PE
                    pst = tp_ps.tile([P, C], F32, tag="tp")
                    for o in range(CC):
                        nc.tensor.transpose(pst[:, ts(o, P)], x_t[:, ts(o, P)], ident[:])
                    nc.vector.tensor_copy(
                        xt_slab[:, :, ts(tt, P)],
                        pst[:].rearrange("p (o n) -> p o n", o=CC),
                    )

                    # q = relu(x @ Wq + bq)   (token-major)
                    q_ps = proj_ps.tile([P, C], F32, tag="proj")
                    for o in range(CC):
                        nc.tensor.matmul(
                            q_ps[:],
                            xt_slab[:, o, ts(tt, P)],
                            wqkv_r[:, o, 0:C],
                            start=(o == 0),
                            stop=False,
                        )
                    nc.tensor.matmul(
                        q_ps[:], ones_r[:], brow_r[:, 0:C], start=False, stop=True
                    )
                    q_sb = qk.tile([P, C], MMD, tag="qk")
                    nc.scalar.activation(
                        q_sb[:], q_ps[:], mybir.ActivationFunctionType.Relu
                    )

                    # k = relu(x @ Wk + bk)
                    k_ps = proj_ps.tile([P, C], F32, tag="proj")
                    for o in range(CC):
                        nc.tensor.matmul(
                            k_ps[:],
                            xt_slab[:, o, ts(tt, P)],
                            wqkv_r[:, o, C : 2 * C],
                            start=(o == 0),
                            stop=False,
                        )
                    nc.tensor.matmul(
                        k_ps[:], ones_r[:], brow_r[:, C : 2 * C], start=False, stop=True
                    )
                    k_sb = qk.tile([P, C], MMD, tag="qk")
                    nc.vector.tensor_scalar_max(k_sb[:], k_ps[:], 0.0)

                    # scores[o] += q[:, o-chunk]^T @ k
                    for o in range(CC):
                        nc.tensor.matmul(
                            scores_ps[o][:],
                            q_sb[:, ts(o, P)],
                            k_sb[:],
                            start=(t == 0),
                            stop=(t == NT - 1),
                        )

                # vT[d, n] = relu(Wv^T x^T + bv)  (channel-major, kept in SBUF)
                for d in range(CC):
                    v_ps = proj_ps.tile([P, C], F32, tag="proj")
                    for o in range(CC):
                        nc.tensor.matmul(
                            v_ps[:, :SLAB],
                            wqkv_r[:, o, ds(2 * C + d * P, P)],
                            xt_slab[:, o, :],
                            start=(o == 0),
                            stop=(o == CC - 1),
                        )
                    nc.scalar.activation(
                        vt_sb[:, d, ts(sl, SLAB)],
                        v_ps[:, :SLAB],
                        mybir.ActivationFunctionType.Relu,
                        bias=bv[:, d : d + 1],
                    )

            # ---- softmax + adjacency gate ------------------------------
            with tc.tile_pool(name="smx", bufs=8) as smx:
                for o in range(CC):
                    smax = smx.tile([P, 1], F32, tag="smax")
                    nc.vector.reduce_max(
                        smax[:], scores_ps[o][:], axis=mybir.AxisListType.X
                    )
                    nbias = smx.tile([P, 1], F32, tag="nbias")
                    nc.vector.tensor_scalar_mul(nbias[:], smax[:], -s)
                    ssum = smx.tile([P, 1], F32, tag="ssum")
                    attn_e = smx.tile([P, C], F32, tag="attn_e")
                    nc.scalar.activation(
                        attn_e[:],
                        scores_ps[o][:],
                        mybir.ActivationFunctionType.Exp,
                        bias=nbias[:],
                        scale=s,
                        accum_out=ssum[:],
                    )
                    rsum = smx.tile([P, 1], F32, tag="rsum")
                    nc.vector.reciprocal(rsum[:], ssum[:])
                    attn_r = smx.tile([P, C], F32, tag="attn_r")
                    nc.vector.tensor_scalar_mul(attn_r[:], attn_e[:], rsum[:])
                    nc.vector.tensor_mul(
                        attn_sb[:, o, :], attn_r[:], adj_sb[:, o, :]
                    )

        # ---- pass 2: y = v @ attn ; out = y @ Wo + bo ------------------
        with (
            tc.tile_pool(name="y_ps", bufs=y_bufs, space="PSUM") as y_ps_pool,
            tc.tile_pool(name="yt", bufs=2) as ytp,
            tc.tile_pool(name="outp", bufs=3) as outp,
        ):
            def emit_yt(sl):
                yt_slab = ytp.tile([P, CC, SLAB], MMD, tag="yT", name=f"yt_{sl}")
                for d in range(CC):
                    y_ps = y_ps_pool.tile([P, C], F32, tag="y", name=f"y_{sl}_{d}")
                    for o in range(CC):
                        nc.tensor.matmul(
                            y_ps[:, :SLAB],
                            attn_sb[:, o, ts(d, P)],
                            vt_sb[:, o, ts(sl, SLAB)],
                            start=(o == 0),
                            stop=(o == CC - 1),
                        )
                    nc.scalar.copy(yt_slab[:, d, :], y_ps[:, :SLAB])
                return yt_slab

            def emit_out(sl, yt_slab):
                for tt in range(TPS):
                    t = sl * TPS + tt
                    o_ps = y_ps_pool.tile([P, C], F32, tag="y", name=f"o_{sl}_{tt}")
                    for d in range(CC):
                        nc.tensor.matmul(
                            o_ps[:],
                            yt_slab[:, d, ts(tt, P)],
                            wo_r[:, d, :],
                            start=(d == 0),
                            stop=(d == CC - 1),
                        )
                    out_sb = outp.tile([P, C], F32, tag="out", name=f"os_{sl}_{tt}")
                    nc.vector.tensor_tensor(
                        out_sb[:], o_ps[:], bo_bc[:], mybir.AluOpType.add
                    )
                    nc.sync.dma_start(out[ts(t, P), :], out_sb[:])

            if pipe_p2:
                prev = None
                for sl in range(NS):
                    yt_slab = emit_yt(sl)
                    if prev is not None:
                        emit_out(sl - 1, prev)
                    prev = yt_slab
                emit_out(NS - 1, prev)
            else:
                for sl in range(NS):
                    emit_out(sl, emit_yt(sl))

    nc.compile()
    return nc


def _get_nc(reps: int = 1, mm_dt=None, **kw):
    key = ("nc", reps, str(mm_dt), tuple(sorted(kw.items())))
    if key not in _CACHE:
        _CACHE[key] = build(reps, mm_dt, **kw)
    return _CACHE[key]


def _run(inputs, trace=False, reps: int = 1, mm_dt=None, **kw):
    nc = _get_nc(reps, mm_dt, **kw)
    x = np.ascontiguousarray(np.asarray(inputs["x"], dtype=np.float32))
    adj = np.ascontiguousarray(np.asarray(inputs["adj"], dtype=np.float32))
    wqkv = np.ascontiguousarray(np.asarray(inputs["Wqkv"], dtype=np.float32))
    bqkv = np.ascontiguousarray(np.asarray(inputs["bqkv"], dtype=np.float32))
    wo = np.ascontiguousarray(np.asarray(inputs["Wo"], dtype=np.float32))
    bo = np.ascontiguousarray(np.asarray(inputs["bo"], dtype=np.float32))

    in_maps = [
        {
            "x": x[b],
            "adj": adj[b],
            "Wqkv": wqkv,
            "bqkv": bqkv,
            "Wo": wo,
            "bo": bo,
        }
        for b in range(B)
    ]
    res = run_bass_kernel_spmd(
        nc, in_maps, core_ids=list(range(B)), trace=trace
    )
    outp = np.stack([res.results[b]["out"] for b in range(B)], axis=0)
    return outp.astype(np.float32), res


def kernel(**inputs) -> np.ndarray:
    out, _ = _run(inputs, trace=False)
    return out



# revision 58
# speedup vs baseline: 4.2050x; 4.2050x over previous
"""ChannelAttention Trainium2 Bass kernel (v2).

Full (unsharded) inputs -> full output. Data-parallel over batch B=8 across
the 8 NeuronCores (one batch element per core, SPMD program, no collectives).

Per-core math (N=4096 tokens, C=512 channels):
    qkv = x @ Wqkv + bqkv ; q,k,v = relu(split(qkv))
    scores = (q^T k) / sqrt(C)           # [C, C] contraction over tokens
    attn = softmax(scores, -1) * adj
    out = v @ (attn @ Wo) + bo           # associativity: v@(attn@Wo)

v2 changes vs v1 (measured on HW: fp32r matmuls run ~4x slower than bf16):
  * matmuls in bf16 (rel err ~2e-3, well under the 2e-2 gate)
  * q/k biases preloaded into PSUM by the Pool engine; projection matmuls
    accumulate with start=False (removes 2 bias matmuls per token tile)
  * x staged as f32r so the PE transpose runs at 1.5 cycles/row
  * pass 2 computes M = attn @ Wo (tiny [C,C]@[C,C]) then out = v @ M,
    replacing the [N,C]@[C,C] y-stage (saves ~57k PE cycles)
  * per-slab transpose-first ordering to hide the xT copy latency
"""

import sys

sys.path.insert(0, "/opt/trn_rl_repo")

from contextlib import ExitStack

import numpy as np

import concourse.bass as bass
import concourse.mybir as mybir
import concourse.tile as tile
from concourse import bacc
from concourse.bass import ds, ts
from concourse.bass_utils import run_bass_kernel_spmd
from concourse.masks import make_identity

# Problem shape (hardcoded per contract).
B, N, C = 8, 4096, 512
P = 128
CC = C // P            # channel chunks (4)
NT = N // P            # token tiles (32)
TPS = 4                # token tiles per slab
NS = NT // TPS         # slabs (8)
SLAB = TPS * P         # tokens per slab (512)

F32 = mybir.dt.float32
F32R = mybir.dt.float32r
BF16 = mybir.dt.bfloat16
F8 = mybir.dt.float8e4

_CACHE = {}


def build(reps: int = 1, mm_dt=None, xt_f32r=0, preload=0, preload_eng=1,
          pipe_scores=1, dma_tp=0, fp8_scores=1, fp8_v=1, fold_tp=0, xt_bf16=0,
          v_split=1, smx_merge=1, tp_bufs=2, proj_bufs=2, qk_bufs=4,
          xin_bufs=3, xtp_bufs=2, o_bufs=3):
    assert not (fold_tp and not dma_tp), "fold_tp requires dma_tp"
    MMD = BF16 if mm_dt is None else mm_dt
    XTD = F32R if xt_f32r else F32
    nc = bacc.Bacc("TRN2", target_bir_lowering=False, debug=False, num_devices=8)

    x = nc.dram_tensor("x", [N, C], F32, kind="ExternalInput").ap()
    adj = nc.dram_tensor("adj", [C, C], F32, kind="ExternalInput").ap()
    wqkv = nc.dram_tensor("Wqkv", [C, 3 * C], F32, kind="ExternalInput").ap()
    bqkv = nc.dram_tensor("bqkv", [3 * C], F32, kind="ExternalInput").ap()
    wo = nc.dram_tensor("Wo", [C, C], F32, kind="ExternalInput").ap()
    bo = nc.dram_tensor("bo", [C], F32, kind="ExternalInput").ap()
    out = nc.dram_tensor("out", [N, C], F32, kind="ExternalOutput").ap()

    s = 1.0 / float(np.sqrt(C))

    def preload_copy(dst, src):
        # preload_eng: 0=Pool, 1=Act, 2=DVE
        if preload_eng == 1:
            nc.scalar.copy(dst, src)
        elif preload_eng == 2:
            nc.vector.tensor_copy(dst, src)
        else:
            nc.gpsimd.tensor_copy(dst, src)

    with tile.TileContext(nc) as tc, ExitStack() as ctx:
        const = ctx.enter_context(tc.tile_pool(name="const", bufs=1))

        # ---- constants -------------------------------------------------
        with tc.tile_pool(name="stage", bufs=1) as stage:
            wqkv_f = stage.tile([P, CC, 3 * C], F32, tag="stage_wqkv")
            nc.sync.dma_start(wqkv_f[:], wqkv.rearrange("(o p) d -> p o d", p=P))
            wqkv_r = const.tile([P, CC, 3 * C], MMD)
            nc.vector.tensor_copy(wqkv_r[:], wqkv_f[:])

            wo_f = stage.tile([P, CC, C], F32, tag="stage_wo")
            nc.sync.dma_start(wo_f[:], wo.rearrange("(o p) d -> p o d", p=P))
            wo_r = const.tile([P, CC, C], MMD)
            if fp8_v:
                # scale Wo so M = attn @ (64*Wo) clears fp8e4m3's denormal
                # range; the out epilogue multiplies by 1/64
                nc.vector.tensor_scalar_mul(wo_r[:], wo_f[:], 64.0)
            else:
                nc.vector.tensor_copy(wo_r[:], wo_f[:])

            brow_f = stage.tile([1, 3 * C], F32, tag="stage_b")
            nc.sync.dma_start(brow_f[:], bqkv[None, :])
            brow = const.tile([1, 2 * C], MMD)
            nc.vector.tensor_copy(brow[:], brow_f[:, 0 : 2 * C])
            borow = stage.tile([1, C], F32, tag="stage_bo")
            nc.sync.dma_start(borow[:], bo[None, :])
            ones_f = stage.tile([1, P], F32, tag="stage_ones")
            nc.gpsimd.memset(ones_f[:], 1.0)
            ones = const.tile([1, P], MMD)
            nc.vector.tensor_copy(ones[:], ones_f[:])

            # broadcast bq/bk/bo to [P, C] once (f32 matmul, outside loop)
            bq_bc = const.tile([P, C], F32)
            bk_bc = const.tile([P, C], F32)
            bo_bc = const.tile([P, C], F32)
            with tc.tile_pool(name="bc_ps", bufs=1, space="PSUM") as bc_pool:
                for dst, src in (
                    (bq_bc, brow_f[:, 0:C]),
                    (bk_bc, brow_f[:, C : 2 * C]),
                    (bo_bc, borow[:, :]),
                ):
                    ps = bc_pool.tile([P, C], F32, tag="bc", name="bc_ps")
                    nc.tensor.matmul(ps[:], ones_f[:], src, start=True, stop=True)
                    nc.vector.tensor_copy(dst[:], ps[:])

        # v-bias, per-partition layout [p, chunk]
        bv = const.tile([P, CC], F32)
        nc.sync.dma_start(bv[:], bqkv[2 * C :].rearrange("(o p) -> p o", p=P))

        ident = const.tile([P, P], F32)
        make_identity(nc, ident[:])
        ident_x = ident[:].bitcast(XTD) if XTD is F32R else ident[:]
        ident_h = const.tile([P, P], MMD)
        nc.vector.tensor_copy(ident_h[:], ident[:])
        # dtype flowing through the tp psum ring (x transposes + attn^T)
        TPD = MMD if xt_bf16 else F32

        adj_sb = const.tile([P, CC, C], F32)
        nc.sync.dma_start(adj_sb[:], adj.rearrange("(o p) d -> p o d", p=P))

        VD = F8 if fp8_v else MMD
        # v^T channel-major; fp8 pairs the CC chunks as [2, CC//2] for DoubleRow
        vt_sb = const.tile([P, 2, CC // 2, N] if fp8_v else [P, CC, N], VD)
        attnT_sb = const.tile([P, CC, C], MMD)   # attn^T (ck-major)
        m_sb = const.tile([P, 2, CC // 2, C] if fp8_v else [P, CC, C], VD)

        scores_pool = ctx.enter_context(
            tc.tile_pool(name="scores", bufs=1, space="PSUM")
        )
        scores_ps = [
            scores_pool.tile([P, C], F32, tag=f"scores{o}", name=f"scores{o}")
            for o in range(CC)
        ]

        rep_ctx = tc.For_i(0, reps, 1) if reps > 1 else None
        if rep_ctx is not None:
            ctx.enter_context(rep_ctx)

        # ---- pass 1: qkv projection + channel scores -------------------
        with (
            tc.tile_pool(name="proj_ps", bufs=proj_bufs, space="PSUM") as proj_ps,
            tc.tile_pool(name="xin", bufs=xin_bufs) as xin,
            tc.tile_pool(name="xtp", bufs=xtp_bufs) as xtp,
            tc.tile_pool(name="qk", bufs=qk_bufs) as qk,
            ExitStack() as p1ctx,
        ):
            tp_ps = (
                proj_ps
                if fold_tp
                else p1ctx.enter_context(
                    tc.tile_pool(name="tp_ps", bufs=tp_bufs, space="PSUM")
                )
            )
            tp_tag = "proj" if fold_tp else "tp"
            for sl in range(NS):
                xt_slab = xtp.tile([P, CC, SLAB], MMD, tag="xT")
                # transpose-first: all 4 tiles of the slab
                for tt in range(TPS):
                    t = sl * TPS + tt
                    if dma_tp == 1:
                        # casting DMA (SWDGE) loads x as bf16 directly
                        xbf = xin.tile([P, C], MMD, tag="xbf")
                        nc.gpsimd.dma_start(xbf[:], x[ts(t, P), :])
                        for o in range(CC):
                            nc.sync.dma_start_transpose(
                                xt_slab[:, o, ts(tt, P)], xbf[:, ts(o, P)]
                            )
                    elif dma_tp == 2:
                        x_t = xin.tile([P, C], F32, tag="x")
                        nc.sync.dma_start(x_t[:], x[ts(t, P), :])
                        xbf = xin.tile([P, C], MMD, tag="xbf")
                        nc.vector.tensor_copy(xbf[:], x_t[:])
                        for o in range(CC):
                            nc.sync.dma_start_transpose(
                                xt_slab[:, o, ts(tt, P)], xbf[:, ts(o, P)]
                            )
                    elif xt_bf16:
                        # convert on Act, transpose in bf16 (1 cycle/row)
                        x_t = xin.tile([P, C], F32, tag="x")
                        nc.sync.dma_start(x_t[:], x[ts(t, P), :])
                        xbf = xin.tile([P, C], MMD, tag="xbf")
                        nc.scalar.copy(xbf[:], x_t[:])
                        pst = tp_ps.tile([P, C], MMD, tag=tp_tag)
                        for o in range(CC):
                            nc.tensor.transpose(
                                pst[:, ts(o, P)], xbf[:, ts(o, P)], ident_h
                            )
                        nc.vector.tensor_copy(
                            xt_slab[:, :, ts(tt, P)],
                            pst[:].rearrange("p (o n) -> p o n", o=CC),
                        )
                    else:
                        x_t = xin.tile([P, C], F32, tag="x")
                        nc.sync.dma_start(x_t[:], x[ts(t, P), :])
                        pst = tp_ps.tile([P, C], XTD, tag=tp_tag)
                        for o in range(CC):
                            xt_chunk = (
                                x_t[:, ts(o, P)].bitcast(F32R)
                                if XTD is F32R
                                else x_t[:, ts(o, P)]
                            )
                            nc.tensor.transpose(pst[:, ts(o, P)], xt_chunk, ident_x)
                        pst_f = pst[:].bitcast(F32) if XTD is F32R else pst[:]
                        nc.vector.tensor_copy(
                            xt_slab[:, :, ts(tt, P)],
                            pst_f.rearrange("p (o n) -> p o n", o=CC),
                        )

                def emit_scores(qs, ks, t):
                    for o in range(CC):
                        nc.tensor.matmul(
                            scores_ps[o][:],
                            qs[:, ts(o, P)],
                            ks[:],
                            start=(t == 0),
                            stop=(t == NT - 1),
                        )

                def emit_scores8(q8s, k8s, pr):
                    for o in range(CC):
                        nc.tensor.matmul(
                            scores_ps[o][:],
                            q8s[:, :, ts(o, P)],
                            k8s[:, :, :],
                            start=(pr == 0),
                            stop=(pr == NT // 2 - 1),
                            perf_mode=mybir.MatmulPerfMode.DoubleRow,
                        )

                pending = None  # scores one tile (or pair) behind
                q8 = k8 = None
                for tt in range(TPS):
                    t = sl * TPS + tt
                    # q = relu(x @ Wq + bq)   (token-major)
                    q_ps = proj_ps.tile([P, C], F32, tag="proj")
                    pre_q = preload in (1, 2)
                    if pre_q:
                        preload_copy(q_ps[:], bq_bc[:])
                    for o in range(CC):
                        nc.tensor.matmul(
                            q_ps[:],
                            xt_slab[:, o, ts(tt, P)],
                            wqkv_r[:, o, 0:C],
                            start=bool(o == 0 and not pre_q),
                            stop=bool(o == CC - 1 and pre_q),
                        )
                    if not pre_q:
                        nc.tensor.matmul(
                            q_ps[:], ones[:], brow[:, 0:C], start=False, stop=True
                        )
                    if fp8_scores:
                        if t % 2 == 0:
                            q8 = qk.tile([P, 2, C], F8, tag="qk8q")
                            k8 = qk.tile([P, 2, C], F8, tag="qk8k")
                        nc.scalar.activation(
                            q8[:, t % 2, :], q_ps[:],
                            mybir.ActivationFunctionType.Relu,
                        )
                    else:
                        q_sb = qk.tile([P, C], MMD, tag="qk")
                        nc.scalar.activation(
                            q_sb[:], q_ps[:], mybir.ActivationFunctionType.Relu
                        )

                    # k = relu(x @ Wk + bk)
                    k_ps = proj_ps.tile([P, C], F32, tag="proj")
                    pre_k = preload == 1
                    if pre_k:
                        preload_copy(k_ps[:], bk_bc[:])
                    for o in range(CC):
                        nc.tensor.matmul(
                            k_ps[:],
                            xt_slab[:, o, ts(tt, P)],
                            wqkv_r[:, o, C : 2 * C],
                            start=bool(o == 0 and not pre_k),
                            stop=bool(o == CC - 1 and pre_k),
                        )
                    if not pre_k:
                        nc.tensor.matmul(
                            k_ps[:], ones[:], brow[:, C : 2 * C], start=False,
                            stop=True,
                        )
                    if fp8_scores:
                        nc.vector.tensor_scalar_max(k8[:, t % 2, :], k_ps[:], 0.0)
                        if t % 2 == 1:
                            if pending is not None:
                                emit_scores8(*pending)
                            pending = (q8, k8, t // 2)
                    else:
                        k_sb = qk.tile([P, C], MMD, tag="qk")
                        nc.vector.tensor_scalar_max(k_sb[:], k_ps[:], 0.0)

                        # scores one tile behind so the relus hide under PE work
                        if not pipe_scores:
                            emit_scores(q_sb, k_sb, t)
                        else:
                            if pending is not None:
                                emit_scores(*pending)
                            pending = (q_sb, k_sb, t)

                # vT[d, n] = relu(Wv^T x^T + bv)  (channel-major, kept in SBUF)
                def emit_v(d):
                    v_ps = proj_ps.tile([P, C], F32, tag="proj")
                    for o in range(CC):
                        nc.tensor.matmul(
                            v_ps[:, :SLAB],
                            wqkv_r[:, o, ds(2 * C + d * P, P)],
                            xt_slab[:, o, :],
                            start=(o == 0),
                            stop=(o == CC - 1),
                        )
                    vt_dst = (
                        vt_sb[:, d % 2, d // 2, ts(sl, SLAB)]
                        if fp8_v
                        else vt_sb[:, d, ts(sl, SLAB)]
                    )
                    nc.scalar.activation(
                        vt_dst,
                        v_ps[:, :SLAB],
                        mybir.ActivationFunctionType.Relu,
                        bias=bv[:, d : d + 1],
                    )

                # split v around the slab's last scores: the first two
                # v-chunks hide the relu latency, the rest follow
                head = (0, 1) if v_split else (0, 1, 2, 3)
                for d in head:
                    emit_v(d)
                if fp8_scores:
                    emit_scores8(*pending)
                    pending = None
                elif pipe_scores:
                    emit_scores(*pending)
                    pending = None
                if v_split:
                    emit_v(2)
                    emit_v(3)

            # ---- softmax + adjacency gate ------------------------------
            attn_g = []
            with (
                tc.tile_pool(name="smx", bufs=4) as smx,
                tc.tile_pool(name="ag", bufs=1) as agp,
            ):
                for o in range(CC):
                    smax = smx.tile([P, 1], F32, tag="smax")
                    nc.vector.reduce_max(
                        smax[:], scores_ps[o][:], axis=mybir.AxisListType.X
                    )
                    nbias = smx.tile([P, 1], F32, tag="nbias")
                    nc.vector.tensor_scalar_mul(nbias[:], smax[:], -s)
                    ssum = smx.tile([P, 1], F32, tag="ssum")
                    attn_e = smx.tile([P, C], F32, tag="attn_e")
                    nc.scalar.activation(
                        attn_e[:],
                        scores_ps[o][:],
                        mybir.ActivationFunctionType.Exp,
                        bias=nbias[:],
                        scale=s,
                        accum_out=ssum[:],
                    )
                    rsum = smx.tile([P, 1], F32, tag="rsum")
                    nc.vector.reciprocal(rsum[:], ssum[:])
                    ag = agp.tile([P, C], TPD, tag=f"attn_g{o}", name=f"attn_g{o}")
                    if smx_merge:
                        nc.vector.scalar_tensor_tensor(
                            ag[:], attn_e[:], rsum[:, 0:1], adj_sb[:, o, :],
                            mybir.AluOpType.mult, mybir.AluOpType.mult,
                        )
                    else:
                        attn_r = smx.tile([P, C], F32, tag="attn_r")
                        nc.vector.tensor_scalar_mul(attn_r[:], attn_e[:], rsum[:])
                        nc.vector.tensor_mul(ag[:], attn_r[:], adj_sb[:, o, :])
                    attn_g.append(ag)

                # ---- attn^T (16 PE block transposes, tp ring) ----------
                id_t = ident_h if TPD is MMD else ident[:]
                for oc in range(CC):
                    at = tp_ps.tile([P, C], TPD, tag=tp_tag)
                    at_f = at[:].bitcast(F32) if TPD is F32R else at[:]
                    for o in range(CC):
                        nc.tensor.transpose(
                            at_f[:, ts(o, P)], attn_g[o][:, ts(oc, P)], id_t
                        )
                    nc.vector.tensor_copy(attnT_sb[:, oc, :], at_f)

                # ---- M = attn @ Wo  (cq-major rows) --------------------
                for o in range(CC):
                    m_ps = proj_ps.tile([P, C], F32, tag="proj")
                    for oc in range(CC):
                        nc.tensor.matmul(
                            m_ps[:],
                            attnT_sb[:, oc, ts(o, P)],
                            wo_r[:, oc, :],
                            start=(oc == 0),
                            stop=(oc == CC - 1),
                        )
                    m_dst = m_sb[:, o % 2, o // 2, :] if fp8_v else m_sb[:, o, :]
                    nc.scalar.copy(m_dst, m_ps[:])

        # ---- pass 2: out = v @ M + bo ----------------------------------
        with (
            tc.tile_pool(name="o_ps", bufs=o_bufs, space="PSUM") as o_ps_pool,
            tc.tile_pool(name="outp", bufs=3) as outp,
        ):
            for t in range(NT):
                o_ps = o_ps_pool.tile([P, C], F32, tag="o", name=f"o_{t}")
                if fp8_v:
                    for o2 in range(CC // 2):
                        nc.tensor.matmul(
                            o_ps[:],
                            vt_sb[:, :, o2, ts(t, P)],
                            m_sb[:, :, o2, :],
                            start=(o2 == 0),
                            stop=(o2 == CC // 2 - 1),
                            perf_mode=mybir.MatmulPerfMode.DoubleRow,
                        )
                else:
                    for o in range(CC):
                        nc.tensor.matmul(
                            o_ps[:],
                            vt_sb[:, o, ts(t, P)],
                            m_sb[:, o, :],
                            start=(o == 0),
                            stop=(o == CC - 1),
                        )
                out_sb = outp.tile([P, C], F32, tag="out", name=f"os_{t}")
                if fp8_v:
                    nc.vector.scalar_tensor_tensor(
                        out_sb[:], o_ps[:], 1.0 / 64.0, bo_bc[:],
                        mybir.AluOpType.mult, mybir.AluOpType.add,
                    )
                else:
                    nc.vector.tensor_tensor(
                        out_sb[:], o_ps[:], bo_bc[:], mybir.AluOpType.add
                    )
                nc.sync.dma_start(out[ts(t, P), :], out_sb[:])

    nc.compile()
    return nc


def _get_nc(reps: int = 1, mm_dt=None, **kw):
    key = ("nc", reps, str(mm_dt), tuple(sorted(kw.items())))
    if key not in _CACHE:
        _CACHE[key] = build(reps, mm_dt, **kw)
    return _CACHE[key]


def _run(inputs, trace=False, reps: int = 1, mm_dt=None, **kw):
    nc = _get_nc(reps, mm_dt, **kw)
    x = np.ascontiguousarray(np.asarray(inputs["x"], dtype=np.float32))
    adj = np.ascontiguousarray(np.asarray(inputs["adj"], dtype=np.float32))
    wqkv = np.ascontiguousarray(np.asarray(inputs["Wqkv"], dtype=np.float32))
    bqkv = np.ascontiguousarray(np.asarray(inputs["bqkv"], dtype=np.float32))
    wo = np.ascontiguousarray(np.asarray(inputs["Wo"], dtype=np.float32))
    bo = np.ascontiguousarray(np.asarray(inputs["bo"], dtype=np.float32))

    in_maps = [
        {
            "x": x[b],
            "adj": adj[b],
            "Wqkv": wqkv,
            "bqkv": bqkv,
            "Wo": wo,
            "bo": bo,
        }
        for b in range(B)
    ]
    res = run_bass_kernel_spmd(
        nc, in_maps, core_ids=list(range(B)), trace=trace
    )
    outp = np.stack([res.results[b]["out"] for b in range(B)], axis=0)
    return outp.astype(np.float32), res


def kernel(**inputs) -> np.ndarray:
    out, _ = _run(inputs, trace=False)
    return out
